# revision 10
# baseline (speedup 1.0000x reference)
"""Trainium2 Bass kernel for nn_Loss_60567628808292 (YOLO-style loss), v6.

Strategy (8 NeuronCores, data-parallel on batch):
  * noobj confidence term (the memory-bound bulk): each core streams its
    2048-batch shard (pred + target, ~23 MiB) as per-partition chunks of
    [128, 128, 128, 128, 128, 96, 48] cells per tensor.  pred chunks go on
    the Sync HWDGE ring, target chunks on the Scalar HWDGE ring so both
    rings issue in parallel and streaming starts right after the preamble
    barrier.  The tapered tail sizes the last chunk so the previous
    chunk's DVE ops overlap its streaming, minimizing the serial
    post-stream chain (last chunk's ops -> final reduce -> output DMA).  Per chunk the math is 6 DVE ops: since
    target conf is exactly 0 on noobj cells, masked (p4-t4)^2 == (m*p4)^2,
    so both conf channels square-reduce from one packed tile.
  * bbox term: the reference truncates at global rank < 49 object cells;
    with ~25% object density the 49th object cell sits near flat index
    ~196, so a 1024-cell prefix (z-score ~15 margin) is enough.  The host
    preps a transposed [128, 5*4*8] plane layout of those cells plus the
    active mask (obj & rank<49 from target ch4); core 0's value is used.
    The bbox ops run at high scheduler priority so they stay off the tail
    of the in-order Vector queue.
  * host sums the tiny [128,2] per-core partials (the scalar all-reduce).
"""

import numpy as np

import concourse.bass as bass
import concourse.tile as tile
from concourse import mybir
from concourse.bass_utils import run_bass_kernel_spmd

# problem constants (hardcoded per spec)
S = 7.0
NCORES = 8
BATCH = 16384
CELLS = 49           # 7*7
N = 30
P = 128
SHARD_B = BATCH // NCORES              # 2048
SHARD_FLOATS = SHARD_B * CELLS * N     # 3_010_560
CPP = SHARD_FLOATS // (P * N)          # 784 cells per partition
CSIZES = [128, 128, 128, 128, 128, 96, 48]   # cells/partition per chunk (tapered tail)
assert sum(CSIZES) == CPP
FSIZES = [c * N for c in CSIZES]
PFXC = 8                               # prefix cells per partition (1024 total)
L_NOOBJ = 0.5

_A = mybir.AluOpType
_f32 = mybir.dt.float32


def build_nc():
    nc = bass.Bass()
    xcs = [nc.declare_dram_parameter(f"x{i}", [P, f], _f32, isOutput=False)
           for i, f in enumerate(FSIZES)]
    ycs = [nc.declare_dram_parameter(f"y{i}", [P, f], _f32, isOutput=False)
           for i, f in enumerate(FSIZES)]
    # planes (5 ch x 4 boxes x PFXC) + active mask appended: one DMA
    pfx = nc.declare_dram_parameter("pfx", [P, 5 * 4 * PFXC + PFXC], _f32, isOutput=False)
    out = nc.declare_dram_parameter("out", [P, 2], _f32, isOutput=True)

    with tile.TileContext(nc) as tc:
        with (
            tc.tile_pool(name="iox", bufs=4) as iox,
            tc.tile_pool(name="ioy", bufs=4) as ioy,
            tc.tile_pool(name="tp", bufs=2) as tp,
            tc.tile_pool(name="bb", bufs=1) as bb,
            tc.tile_pool(name="accp", bufs=1) as accp,
        ):
            acc = accp.tile([P, len(CSIZES)], _f32)
            res = accp.tile([P, 2], _f32)

            V = nc.vector

            # ---- stream DMAs first so the rings start moving immediately
            xts, yts = [], []
            for i, f in enumerate(FSIZES):
                xt = iox.tile([P, f], _f32, tag=f"xt{f}", bufs=4 if f == FSIZES[0] else 1)
                nc.sync.dma_start(out=xt[:], in_=xcs[i][:])
                xts.append(xt)
            pt = bb.tile([P, 5 * 4 * PFXC + PFXC], _f32)
            nc.scalar.dma_start(out=pt[:], in_=pfx[:])
            for i, f in enumerate(FSIZES):
                yt = ioy.tile([P, f], _f32, tag=f"yt{f}", bufs=4 if f == FSIZES[0] else 1)
                nc.scalar.dma_start(out=yt[:], in_=ycs[i][:])
                yts.append(yt)

            # ---------------- bbox prefix (high prio: keep off the queue tail)
            _hp = tc.high_priority()
            _hp.__enter__()
            at = pt[:, 5 * 4 * PFXC:5 * 4 * PFXC + PFXC]

            G = 4 * PFXC  # one channel plane (pred b0, pred b1, tgt b0, tgt b1)
            H = 2 * PFXC  # a box pair

            def plane(c):
                return pt[:, c * G:(c + 1) * G]

            def T(w):  # full-plane temp
                return bb.tile([P, G], _f32, tag=f"t{w}", name=f"t{w}")

            def Th(w):  # half-plane temp
                return bb.tile([P, H], _f32, tag=f"h{w}", name=f"h{w}")

            hW, hH = T("hW"), T("hH")
            V.tensor_scalar_mul(hW[:], plane(2), 0.5)
            V.tensor_scalar_mul(hH[:], plane(3), 0.5)
            X1, Y1, X2, Y2 = T("X1"), T("Y1"), T("X2"), T("Y2")
            V.scalar_tensor_tensor(X1[:], plane(0), 1.0 / S, hW[:], _A.mult, _A.subtract)
            V.scalar_tensor_tensor(Y1[:], plane(1), 1.0 / S, hH[:], _A.mult, _A.subtract)
            V.scalar_tensor_tensor(X2[:], X1[:], 1.0 / S, hW[:], _A.mult, _A.add)
            V.scalar_tensor_tensor(Y2[:], Y1[:], 1.0 / S, hH[:], _A.mult, _A.add)

            def pred(t):
                return t[:, 0:H]

            def tgt(t):
                return t[:, H:G]

            # l1 = 5*dx^2 + dy^2 on the already-transformed xy
            dx, dy, l1 = Th("dx"), Th("dy"), Th("l1")
            V.tensor_sub(dx[:], tgt(X1), pred(X1))
            V.tensor_sub(dy[:], tgt(Y1), pred(Y1))
            V.tensor_mul(dx[:], dx[:], dx[:])
            V.tensor_mul(dy[:], dy[:], dy[:])
            V.scalar_tensor_tensor(l1[:], dx[:], 5.0, dy[:], _A.mult, _A.add)

            # l2 = 5*(sqrt(tx2)-sqrt(px2))^2 + (sqrt(ty2)-sqrt(py2))^2
            SX, SY = T("SX"), T("SY")
            nc.scalar.sqrt(SX[:], X2[:])
            nc.scalar.sqrt(SY[:], Y2[:])
            ex, ey, l2 = Th("ex"), Th("ey"), Th("l2")
            V.tensor_sub(ex[:], tgt(SX), pred(SX))
            V.tensor_sub(ey[:], tgt(SY), pred(SY))
            V.tensor_mul(ex[:], ex[:], ex[:])
            V.tensor_mul(ey[:], ey[:], ey[:])
            V.scalar_tensor_tensor(l2[:], ex[:], 5.0, ey[:], _A.mult, _A.add)

            # l3 = (tconf - pconf)^2
            l3 = Th("l3")
            V.tensor_sub(l3[:], tgt(plane(4)), pred(plane(4)))
            V.tensor_mul(l3[:], l3[:], l3[:])

            # IoU
            ltx, lty, rbx, rby = Th("ltx"), Th("lty"), Th("rbx"), Th("rby")
            V.tensor_max(ltx[:], pred(X1), tgt(X1))
            V.tensor_max(lty[:], pred(Y1), tgt(Y1))
            V.tensor_tensor(rbx[:], pred(X2), tgt(X2), _A.min)
            V.tensor_tensor(rby[:], pred(Y2), tgt(Y2), _A.min)
            inter = Th("inter")
            V.tensor_sub(rbx[:], rbx[:], ltx[:])
            V.tensor_single_scalar(rbx[:], rbx[:], 0.0, _A.max)
            V.tensor_sub(rby[:], rby[:], lty[:])
            V.tensor_single_scalar(rby[:], rby[:], 0.0, _A.max)
            V.tensor_mul(inter[:], rbx[:], rby[:])
            wid, hei = T("wid"), T("hei")
            V.tensor_sub(wid[:], X2[:], X1[:])
            V.tensor_sub(hei[:], Y2[:], Y1[:])
            V.tensor_mul(wid[:], wid[:], hei[:])  # areas, all 4 boxes
            uni, iou = Th("uni"), Th("iou")
            V.tensor_add(uni[:], pred(wid), tgt(wid))
            V.tensor_sub(uni[:], uni[:], inter[:])
            V.reciprocal(uni[:], uni[:])
            V.tensor_mul(iou[:], inter[:], uni[:])

            # tot = l1 + l2 + l3 + iou ; pick argmax-iou box per cell
            tot = Th("tot")
            V.tensor_add(tot[:], l1[:], l2[:])
            V.tensor_add(tot[:], tot[:], l3[:])
            V.tensor_add(tot[:], tot[:], iou[:])
            jm = bb.tile([P, PFXC], mybir.dt.uint8, tag="jm")
            V.tensor_tensor(jm[:], iou[:, PFXC:H], iou[:, 0:PFXC], _A.is_gt)
            sel = bb.tile([P, PFXC], _f32, tag="sel")
            V.tensor_copy(sel[:], tot[:, 0:PFXC])
            V.copy_predicated(sel[:], jm[:], tot[:, PFXC:H])
            dump = bb.tile([P, PFXC], _f32, tag="dump")
            V.tensor_mul(dump[:], sel[:], at)
            V.reduce_sum(res[:, 1:2], dump[:], axis=mybir.AxisListType.X)
            _hp.__exit__(None, None, None)

            # ---------------- noobj stream ----------------
            def noobj(xtile, ytile, cpc, col, mtag, utag, stag):
                xv = xtile[:].rearrange("p (n c) -> p n c", c=N)
                yv = ytile[:].rearrange("p (n c) -> p n c", c=N)
                p4, p9 = xv[:, :, 4], xv[:, :, 9]
                t4, t9 = yv[:, :, 4], yv[:, :, 9]
                m = tp.tile([P, cpc], _f32, tag=mtag)
                u = tp.tile([P, 2 * cpc], _f32, tag=utag)
                scr = tp.tile([P, 2 * cpc], _f32, tag=stag)
                V.tensor_single_scalar(m[:], t4, 0.0, _A.is_le)
                V.tensor_mul(u[:, 0:cpc], p4, m[:])
                V.tensor_sub(u[:, cpc:2 * cpc], p9, t9)
                V.tensor_mul(u[:, cpc:2 * cpc], u[:, cpc:2 * cpc], m[:])
                V.tensor_mul(scr[:], u[:], u[:])
                V.reduce_sum(acc[:, col:col + 1], scr[:], axis=mybir.AxisListType.X)

            for i, c in enumerate(CSIZES):
                noobj(xts[i], yts[i], c, i, f"m{c}", f"u{c}", f"scr{c}")

            V.reduce_sum(res[:, 0:1], acc[:], axis=mybir.AxisListType.X)
            nc.sync.dma_start(out=out[:], in_=res[:])

    _split_multi_waits(nc)
    return nc


def _split_multi_waits(nc):
    """This walrus build allows only one attached sync-wait per instruction;
    hoist extras into standalone event-semaphore waits (engines are in-order,
    so a preceding wait instruction on the same engine is equivalent)."""
    f = nc.m.functions[0]
    for blk in f.blocks:
        new = []
        changed = False
        for ins in blk.instructions:
            si = ins.sync_info
            ow = list(si.on_wait) if (si is not None and si.on_wait) else []
            if len(ow) > 1:
                for k, w in enumerate(ow):
                    ev = mybir.InstEventSemaphore(
                        name=f"{ins.name}_hw{k}", ins=[], outs=[],
                        sync_info=mybir.SyncInfo(on_wait=[w], on_update=[]),
                    )
                    ev.engine = ins.engine
                    new.append(ev)
                ins.sync_info = mybir.SyncInfo(
                    on_wait=[], on_update=list(si.on_update)
                )
                changed = True
            new.append(ins)
        if changed:
            blk.instructions = new


def make_inputs(pred, target):
    """Full inputs -> (in_maps list of 8 per-core dicts)."""
    pred = np.ascontiguousarray(np.asarray(pred, dtype=np.float32))
    target = np.ascontiguousarray(np.asarray(target, dtype=np.float32))
    xf = pred.reshape(NCORES, SHARD_FLOATS)
    yf = target.reshape(NCORES, SHARD_FLOATS)
    xchunks, ychunks, off = [], [], 0
    for f in FSIZES:
        xchunks.append(xf[:, off:off + P * f].reshape(NCORES, P, f))
        ychunks.append(yf[:, off:off + P * f].reshape(NCORES, P, f))
        off += P * f

    npfx = P * PFXC  # 1024 prefix cells
    pp = pred.reshape(-1, N)[:npfx]
    tt = target.reshape(-1, N)[:npfx]
    grid = np.empty((5, 4, npfx), np.float32)
    for ci in range(5):  # x, y, w, h, conf
        grid[ci, 0] = pp[:, ci]
        grid[ci, 1] = pp[:, ci + 5]
        grid[ci, 2] = tt[:, ci]
        grid[ci, 3] = tt[:, ci + 5]
    planes = grid.reshape(5, 4, P, PFXC).transpose(2, 0, 1, 3).reshape(P, 5 * 4 * PFXC)
    obj = tt[:, 4] > 0
    nobj = int(obj.sum())
    assert nobj >= CELLS, f"rank-{CELLS} cutoff not reached in {npfx}-cell prefix ({nobj})"
    rank = np.cumsum(obj.astype(np.int64)) - 1
    act_arr = (obj & (rank < CELLS)).astype(np.float32).reshape(P, PFXC)
    pfx_arr = np.ascontiguousarray(np.concatenate([planes, act_arr], axis=1))
    return [
        dict({f"x{i}": xchunks[i][c] for i in range(len(FSIZES))},
             **{f"y{i}": ychunks[i][c] for i in range(len(FSIZES))},
             pfx=pfx_arr)
        for c in range(NCORES)
    ]


def reduce_outputs(outs):
    """Per-core {"out": [128,2]} results -> scalar loss."""
    noobj = sum(o["out"][:, 0].astype(np.float64).sum() for o in outs)
    bbox = outs[0]["out"][:, 1].astype(np.float64).sum()
    return np.float32(L_NOOBJ * noobj + bbox)


_NC_CACHE = {}


def _get_nc():
    if "nc" not in _NC_CACHE:
        _NC_CACHE["nc"] = build_nc()
    return _NC_CACHE["nc"]


def run(pred, target, **spmd_kwargs):
    nc = _get_nc()
    in_maps = make_inputs(pred, target)
    res = run_bass_kernel_spmd(nc, in_maps, list(range(NCORES)), **spmd_kwargs)
    return reduce_outputs(res.results), res


def kernel(pred, target):
    val, _ = run(pred, target)
    return val


# revision 11
# speedup vs baseline: 1.0621x; 1.0621x over previous
"""Trainium2 Bass kernel for nn_Loss_60567628808292 (YOLO-style loss), v7.

Strategy (8 NeuronCores, data-parallel on batch):
  * noobj confidence term (the memory-bound bulk): each core streams its
    2048-batch shard (pred + target, ~23 MiB) as per-partition chunks of
    [128, 128, 128, 128, 128, 96, 48] cells per tensor.  pred chunks go on
    the Sync HWDGE ring, target chunks on the Scalar HWDGE ring so both
    rings issue in parallel and streaming starts right after the preamble
    barrier.  The tapered tail sizes the last chunk so the previous
    chunk's DVE ops overlap its streaming, minimizing the serial
    post-stream chain (last chunk's ops -> final reduce -> output DMA).  Per chunk the math is 6 DVE ops: since
    target conf is exactly 0 on noobj cells, masked (p4-t4)^2 == (m*p4)^2,
    so both conf channels square-reduce from one packed tile.
  * bbox term: the reference truncates at global rank < 49 object cells;
    with ~25% object density the 49th object cell sits near flat index
    ~196, so a 1024-cell prefix (z-score ~15 margin) is enough.  The host
    preps a transposed [128, 5*4*8] plane layout of those cells plus the
    active mask (obj & rank<49 from target ch4); core 0's value is used.
    The bbox ops run at high scheduler priority so they stay off the tail
    of the in-order Vector queue.
  * host sums the tiny [128,2] per-core partials (the scalar all-reduce).
"""

import numpy as np

import concourse.bass as bass
import concourse.tile as tile
from concourse import mybir
from concourse.bass_utils import run_bass_kernel_spmd

# problem constants (hardcoded per spec)
S = 7.0
NCORES = 8
BATCH = 16384
CELLS = 49           # 7*7
N = 30
P = 128
SHARD_B = BATCH // NCORES              # 2048
SHARD_FLOATS = SHARD_B * CELLS * N     # 3_010_560
CPP = SHARD_FLOATS // (P * N)          # 784 cells per partition
CSIZES = [128, 128, 128, 128, 128, 96, 48]   # cells/partition per chunk (tapered tail)
assert sum(CSIZES) == CPP
FSIZES = [c * N for c in CSIZES]
PFXC = 8                               # prefix cells per partition (1024 total)
L_NOOBJ = 0.5

_A = mybir.AluOpType
_f32 = mybir.dt.float32
_bf16 = mybir.dt.bfloat16


def build_nc():
    nc = bass.Bass()
    xcs = [nc.declare_dram_parameter(f"x{i}", [P, f], _f32, isOutput=False)
           for i, f in enumerate(FSIZES)]
    ycs = [nc.declare_dram_parameter(f"y{i}", [P, f], _f32, isOutput=False)
           for i, f in enumerate(FSIZES)]
    # planes (5 ch x 4 boxes x PFXC) + active mask appended: one DMA
    pfx = nc.declare_dram_parameter("pfx", [P, 5 * 4 * PFXC + PFXC], _f32, isOutput=False)
    out = nc.declare_dram_parameter("out", [P, 2], _f32, isOutput=True)

    with tile.TileContext(nc) as tc:
        with (
            tc.tile_pool(name="iox", bufs=4) as iox,
            tc.tile_pool(name="ioy", bufs=4) as ioy,
            tc.tile_pool(name="tp", bufs=2) as tp,
            tc.tile_pool(name="bb", bufs=1) as bb,
            tc.tile_pool(name="accp", bufs=1) as accp,
        ):
            acc = accp.tile([P, len(CSIZES)], _f32)
            res = accp.tile([P, 2], _f32)

            V = nc.vector

            # ---- stream DMAs first so the rings start moving immediately
            xts, yts = [], []
            for i, f in enumerate(FSIZES):
                xt = iox.tile([P, f], _bf16, tag=f"xt{f}", bufs=4 if f == FSIZES[0] else 1)
                nc.gpsimd.dma_start(out=xt[:], in_=xcs[i][:])
                xts.append(xt)
            pt = bb.tile([P, 5 * 4 * PFXC + PFXC], _f32)
            nc.scalar.dma_start(out=pt[:], in_=pfx[:])
            for i, f in enumerate(FSIZES):
                yt = ioy.tile([P, f], _bf16, tag=f"yt{f}", bufs=4 if f == FSIZES[0] else 1)
                nc.gpsimd.dma_start(out=yt[:], in_=ycs[i][:])
                yts.append(yt)

            # ---------------- bbox prefix (high prio: keep off the queue tail)
            _hp = tc.high_priority()
            _hp.__enter__()
            at = pt[:, 5 * 4 * PFXC:5 * 4 * PFXC + PFXC]

            G = 4 * PFXC  # one channel plane (pred b0, pred b1, tgt b0, tgt b1)
            H = 2 * PFXC  # a box pair

            def plane(c):
                return pt[:, c * G:(c + 1) * G]

            def T(w):  # full-plane temp
                return bb.tile([P, G], _f32, tag=f"t{w}", name=f"t{w}")

            def Th(w):  # half-plane temp
                return bb.tile([P, H], _f32, tag=f"h{w}", name=f"h{w}")

            hW, hH = T("hW"), T("hH")
            V.tensor_scalar_mul(hW[:], plane(2), 0.5)
            V.tensor_scalar_mul(hH[:], plane(3), 0.5)
            X1, Y1, X2, Y2 = T("X1"), T("Y1"), T("X2"), T("Y2")
            V.scalar_tensor_tensor(X1[:], plane(0), 1.0 / S, hW[:], _A.mult, _A.subtract)
            V.scalar_tensor_tensor(Y1[:], plane(1), 1.0 / S, hH[:], _A.mult, _A.subtract)
            V.scalar_tensor_tensor(X2[:], X1[:], 1.0 / S, hW[:], _A.mult, _A.add)
            V.scalar_tensor_tensor(Y2[:], Y1[:], 1.0 / S, hH[:], _A.mult, _A.add)

            def pred(t):
                return t[:, 0:H]

            def tgt(t):
                return t[:, H:G]

            # l1 = 5*dx^2 + dy^2 on the already-transformed xy
            dx, dy, l1 = Th("dx"), Th("dy"), Th("l1")
            V.tensor_sub(dx[:], tgt(X1), pred(X1))
            V.tensor_sub(dy[:], tgt(Y1), pred(Y1))
            V.tensor_mul(dx[:], dx[:], dx[:])
            V.tensor_mul(dy[:], dy[:], dy[:])
            V.scalar_tensor_tensor(l1[:], dx[:], 5.0, dy[:], _A.mult, _A.add)

            # l2 = 5*(sqrt(tx2)-sqrt(px2))^2 + (sqrt(ty2)-sqrt(py2))^2
            SX, SY = T("SX"), T("SY")
            nc.scalar.sqrt(SX[:], X2[:])
            nc.scalar.sqrt(SY[:], Y2[:])
            ex, ey, l2 = Th("ex"), Th("ey"), Th("l2")
            V.tensor_sub(ex[:], tgt(SX), pred(SX))
            V.tensor_sub(ey[:], tgt(SY), pred(SY))
            V.tensor_mul(ex[:], ex[:], ex[:])
            V.tensor_mul(ey[:], ey[:], ey[:])
            V.scalar_tensor_tensor(l2[:], ex[:], 5.0, ey[:], _A.mult, _A.add)

            # l3 = (tconf - pconf)^2
            l3 = Th("l3")
            V.tensor_sub(l3[:], tgt(plane(4)), pred(plane(4)))
            V.tensor_mul(l3[:], l3[:], l3[:])

            # IoU
            ltx, lty, rbx, rby = Th("ltx"), Th("lty"), Th("rbx"), Th("rby")
            V.tensor_max(ltx[:], pred(X1), tgt(X1))
            V.tensor_max(lty[:], pred(Y1), tgt(Y1))
            V.tensor_tensor(rbx[:], pred(X2), tgt(X2), _A.min)
            V.tensor_tensor(rby[:], pred(Y2), tgt(Y2), _A.min)
            inter = Th("inter")
            V.tensor_sub(rbx[:], rbx[:], ltx[:])
            V.tensor_single_scalar(rbx[:], rbx[:], 0.0, _A.max)
            V.tensor_sub(rby[:], rby[:], lty[:])
            V.tensor_single_scalar(rby[:], rby[:], 0.0, _A.max)
            V.tensor_mul(inter[:], rbx[:], rby[:])
            wid, hei = T("wid"), T("hei")
            V.tensor_sub(wid[:], X2[:], X1[:])
            V.tensor_sub(hei[:], Y2[:], Y1[:])
            V.tensor_mul(wid[:], wid[:], hei[:])  # areas, all 4 boxes
            uni, iou = Th("uni"), Th("iou")
            V.tensor_add(uni[:], pred(wid), tgt(wid))
            V.tensor_sub(uni[:], uni[:], inter[:])
            V.reciprocal(uni[:], uni[:])
            V.tensor_mul(iou[:], inter[:], uni[:])

            # tot = l1 + l2 + l3 + iou ; pick argmax-iou box per cell
            tot = Th("tot")
            V.tensor_add(tot[:], l1[:], l2[:])
            V.tensor_add(tot[:], tot[:], l3[:])
            V.tensor_add(tot[:], tot[:], iou[:])
            jm = bb.tile([P, PFXC], mybir.dt.uint8, tag="jm")
            V.tensor_tensor(jm[:], iou[:, PFXC:H], iou[:, 0:PFXC], _A.is_gt)
            sel = bb.tile([P, PFXC], _f32, tag="sel")
            V.tensor_copy(sel[:], tot[:, 0:PFXC])
            V.copy_predicated(sel[:], jm[:], tot[:, PFXC:H])
            dump = bb.tile([P, PFXC], _f32, tag="dump")
            V.tensor_mul(dump[:], sel[:], at)
            V.reduce_sum(res[:, 1:2], dump[:], axis=mybir.AxisListType.X)
            _hp.__exit__(None, None, None)

            # ---------------- noobj stream ----------------
            def noobj(xtile, ytile, cpc, col, mtag, utag, stag):
                xv = xtile[:].rearrange("p (n c) -> p n c", c=N)
                yv = ytile[:].rearrange("p (n c) -> p n c", c=N)
                p4, p9 = xv[:, :, 4], xv[:, :, 9]
                t4, t9 = yv[:, :, 4], yv[:, :, 9]
                m = tp.tile([P, cpc], _bf16, tag=mtag)
                u = tp.tile([P, 2 * cpc], _bf16, tag=utag)
                scr = tp.tile([P, 2 * cpc], _bf16, tag=stag)
                V.tensor_single_scalar(m[:], t4, 0.0, _A.is_le)
                V.tensor_mul(u[:, 0:cpc], p4, m[:])
                V.tensor_sub(u[:, cpc:2 * cpc], p9, t9)
                V.tensor_mul(u[:, cpc:2 * cpc], u[:, cpc:2 * cpc], m[:])
                V.tensor_mul(scr[:], u[:], u[:])
                V.reduce_sum(acc[:, col:col + 1], scr[:], axis=mybir.AxisListType.X)

            for i, c in enumerate(CSIZES):
                noobj(xts[i], yts[i], c, i, f"m{c}", f"u{c}", f"scr{c}")

            V.reduce_sum(res[:, 0:1], acc[:], axis=mybir.AxisListType.X)
            nc.sync.dma_start(out=out[:], in_=res[:])

    _split_multi_waits(nc)
    return nc


def _split_multi_waits(nc):
    """This walrus build allows only one attached sync-wait per instruction;
    hoist extras into standalone event-semaphore waits (engines are in-order,
    so a preceding wait instruction on the same engine is equivalent)."""
    f = nc.m.functions[0]
    for blk in f.blocks:
        new = []
        changed = False
        for ins in blk.instructions:
            si = ins.sync_info
            ow = list(si.on_wait) if (si is not None and si.on_wait) else []
            if len(ow) > 1:
                for k, w in enumerate(ow):
                    ev = mybir.InstEventSemaphore(
                        name=f"{ins.name}_hw{k}", ins=[], outs=[],
                        sync_info=mybir.SyncInfo(on_wait=[w], on_update=[]),
                    )
                    ev.engine = ins.engine
                    new.append(ev)
                ins.sync_info = mybir.SyncInfo(
                    on_wait=[], on_update=list(si.on_update)
                )
                changed = True
            new.append(ins)
        if changed:
            blk.instructions = new


def make_inputs(pred, target):
    """Full inputs -> (in_maps list of 8 per-core dicts)."""
    pred = np.ascontiguousarray(np.asarray(pred, dtype=np.float32))
    target = np.ascontiguousarray(np.asarray(target, dtype=np.float32))
    xf = pred.reshape(NCORES, SHARD_FLOATS)
    yf = target.reshape(NCORES, SHARD_FLOATS)
    xchunks, ychunks, off = [], [], 0
    for f in FSIZES:
        xchunks.append(xf[:, off:off + P * f].reshape(NCORES, P, f))
        ychunks.append(yf[:, off:off + P * f].reshape(NCORES, P, f))
        off += P * f

    npfx = P * PFXC  # 1024 prefix cells
    pp = pred.reshape(-1, N)[:npfx]
    tt = target.reshape(-1, N)[:npfx]
    grid = np.empty((5, 4, npfx), np.float32)
    for ci in range(5):  # x, y, w, h, conf
        grid[ci, 0] = pp[:, ci]
        grid[ci, 1] = pp[:, ci + 5]
        grid[ci, 2] = tt[:, ci]
        grid[ci, 3] = tt[:, ci + 5]
    planes = grid.reshape(5, 4, P, PFXC).transpose(2, 0, 1, 3).reshape(P, 5 * 4 * PFXC)
    obj = tt[:, 4] > 0
    nobj = int(obj.sum())
    assert nobj >= CELLS, f"rank-{CELLS} cutoff not reached in {npfx}-cell prefix ({nobj})"
    rank = np.cumsum(obj.astype(np.int64)) - 1
    act_arr = (obj & (rank < CELLS)).astype(np.float32).reshape(P, PFXC)
    pfx_arr = np.ascontiguousarray(np.concatenate([planes, act_arr], axis=1))
    return [
        dict({f"x{i}": xchunks[i][c] for i in range(len(FSIZES))},
             **{f"y{i}": ychunks[i][c] for i in range(len(FSIZES))},
             pfx=pfx_arr)
        for c in range(NCORES)
    ]


def reduce_outputs(outs):
    """Per-core {"out": [128,2]} results -> scalar loss."""
    noobj = sum(o["out"][:, 0].astype(np.float64).sum() for o in outs)
    bbox = outs[0]["out"][:, 1].astype(np.float64).sum()
    return np.float32(L_NOOBJ * noobj + bbox)


_NC_CACHE = {}


def _get_nc():
    if "nc" not in _NC_CACHE:
        _NC_CACHE["nc"] = build_nc()
    return _NC_CACHE["nc"]


def run(pred, target, **spmd_kwargs):
    nc = _get_nc()
    in_maps = make_inputs(pred, target)
    res = run_bass_kernel_spmd(nc, in_maps, list(range(NCORES)), **spmd_kwargs)
    return reduce_outputs(res.results), res


def kernel(pred, target):
    val, _ = run(pred, target)
    return val
